# revision 4
# baseline (speedup 1.0000x reference)
"""AdaptiveMemoryBank kernel for 8 TRN2 NeuronCores.

Data-parallel over tokens: B*S = 16384 tokens are split into 8 shards of
2048 rows; each core holds the full weight set and computes its shard.

Per-core pipeline (feature-major activation spine, bf16 matmuls):
  pass 1 (selector): XT = gather-transpose(x); hid = relu(scale-mixed
    split-bf16 matmul); logits = split-bf16 matmul -> argmax masks.
    split-bf16 (x_hi@w_hi + x_lo@w_hi + x_hi@w_lo) reproduces the fp32
    argmax exactly (0 flips on the graded input) at bf16 speed.
  pass 2 (branches): c = C@xT, a = A@cT (feature-major), out_k = aT.T@D
    (token-major) with all per-layer biases folded into one effective
    bias per branch (b_eff = D(A bc + ba) + bd) applied via a K=1 matmul;
    per-token select out = m0*x + m1*out1 + m2*out2 with masks as
    per-partition scalars.
"""

import sys, os, types, time

sys.path.insert(0, "/opt/trn_rl_repo")

# Provide the antenv.axon_hooks module the container's antenv stub lacks so
# run_bass_kernel_spmd(trace=True) can capture NTFF timing through axon.
if "antenv.axon_hooks" not in sys.modules:
    _hooks_mod = types.ModuleType("antenv.axon_hooks")
    _hooks_mod._hook = None

    def _set_hook(h):
        _hooks_mod._hook = h

    def _get_hook():
        return _hooks_mod._hook

    _hooks_mod.set_axon_ntff_profile_hook = _set_hook
    _hooks_mod.get_axon_ntff_profile_hook = _get_hook
    sys.modules["antenv.axon_hooks"] = _hooks_mod
    try:
        from trn_agent_boot.trn_boot import _ntff_profile_via_ctypes

        _set_hook(_ntff_profile_via_ctypes("/opt/axon/libaxon_pjrt.so"))
    except Exception:
        pass

import numpy as np
import ml_dtypes

import concourse.bass as bass
import concourse.bacc as bacc
import concourse.tile as tile
import concourse.mybir as mybir
from concourse.bass_utils import run_bass_kernel_spmd

BF16 = mybir.dt.bfloat16
F32 = mybir.dt.float32
I16 = mybir.dt.int16

NCORES = 8
H = 2048
T = 2048          # tokens per core
TC = 256          # tokens per chunk
NCHUNK = T // TC
P = 128

last_exec_time_ns = None


def _bf(x):
    return np.asarray(x, np.float32).astype(ml_dtypes.bfloat16)


def _mm_flags(i, n):
    return dict(start=(i == 0), stop=(i == n - 1))


def build_nc():
    nc = bacc.Bacc(None, target_bir_lowering=False, debug=False)

    d_xhi = nc.dram_tensor("xhi", [T, H], BF16, kind="ExternalInput")
    d_xlo = nc.dram_tensor("xlo", [T, H], BF16, kind="ExternalInput")
    d_freq = nc.dram_tensor("freqr", [1, T], F32, kind="ExternalInput")
    d_imp = nc.dram_tensor("impr", [1, T], F32, kind="ExternalInput")
    d_s1h = nc.dram_tensor("ws1h", [H, 512], BF16, kind="ExternalInput")
    d_s1l = nc.dram_tensor("ws1l", [H, 512], BF16, kind="ExternalInput")
    d_s2h = nc.dram_tensor("ws2h", [512, 3], BF16, kind="ExternalInput")
    d_s2l = nc.dram_tensor("ws2l", [512, 3], BF16, kind="ExternalInput")
    d_wc1 = nc.dram_tensor("wc1", [H, 1024], BF16, kind="ExternalInput")
    d_wa1 = nc.dram_tensor("wa1", [1024, 1024], BF16, kind="ExternalInput")
    d_wd1 = nc.dram_tensor("wd1", [1024, H], BF16, kind="ExternalInput")
    d_wc2 = nc.dram_tensor("wc2", [H, 512], BF16, kind="ExternalInput")
    d_wa2 = nc.dram_tensor("wa2", [512, 512], BF16, kind="ExternalInput")
    d_wd2 = nc.dram_tensor("wd2", [512, H], BF16, kind="ExternalInput")
    d_bs1 = nc.dram_tensor("bs1", [P, 4], F32, kind="ExternalInput")
    d_b2bc = nc.dram_tensor("b2bc", [P, 3], F32, kind="ExternalInput")
    d_be1 = nc.dram_tensor("be1", [1, H], BF16, kind="ExternalInput")
    d_be2 = nc.dram_tensor("be2", [1, H], BF16, kind="ExternalInput")
    d_ones = nc.dram_tensor("onesb", [1, P], BF16, kind="ExternalInput")
    d_onesf = nc.dram_tensor("onesf", [1, P], F32, kind="ExternalInput")
    d_ident = nc.dram_tensor("ident", [P, P], BF16, kind="ExternalInput")
    d_gidx = nc.dram_tensor("gidx", [P, T // 16], I16, kind="ExternalInput")
    d_out = nc.dram_tensor("out", [T, H], F32, kind="ExternalOutput")

    with tile.TileContext(nc) as tc:
        with tc.tile_pool(name="persist", bufs=1) as pp:
            gidx = pp.tile([P, T // 16], I16)
            nc.sync.dma_start(out=gidx[:], in_=d_gidx[:, :])
            onesb = pp.tile([1, P], BF16)
            nc.sync.dma_start(out=onesb[:], in_=d_ones[:, :])
            onesf = pp.tile([1, P], F32)
            nc.sync.dma_start(out=onesf[:], in_=d_onesf[:, :])
            ident = pp.tile([P, P], BF16)
            nc.sync.dma_start(out=ident[:], in_=d_ident[:, :])
            bs1 = pp.tile([P, 4], F32)
            nc.sync.dma_start(out=bs1[:], in_=d_bs1[:, :])
            b2bc = pp.tile([P, 3], F32)
            nc.sync.dma_start(out=b2bc[:], in_=d_b2bc[:, :])
            be1 = pp.tile([1, H], BF16)
            nc.sync.dma_start(out=be1[:], in_=d_be1[:, :])
            be2 = pp.tile([1, H], BF16)
            nc.sync.dma_start(out=be2[:], in_=d_be2[:, :])
            freqr = pp.tile([1, T], F32)
            nc.sync.dma_start(out=freqr[:], in_=d_freq[:, :])
            impr = pp.tile([1, T], F32)
            nc.sync.dma_start(out=impr[:], in_=d_imp[:, :])
            s2h = pp.tile([P, 4, 3], BF16)
            nc.sync.dma_start(out=s2h[:], in_=d_s2h[:, :].rearrange("(f p) n -> p f n", p=P))
            s2l = pp.tile([P, 4, 3], BF16)
            nc.sync.dma_start(out=s2l[:], in_=d_s2l[:, :].rearrange("(f p) n -> p f n", p=P))
            # per-token one-hot expert masks, [128, tok_tile, 3] f32
            masks = pp.tile([P, T // P, 3], F32)

            # ---------------- pass 1: selector ----------------
            with tc.tile_pool(name="selw", bufs=1) as sw, \
                 tc.tile_pool(name="selact", bufs=2) as sa, \
                 tc.tile_pool(name="selps", bufs=2, space="PSUM") as sp, \
                 tc.tile_pool(name="selps1", bufs=1, space="PSUM") as sp1:
                s1h = sw.tile([P, 16, 512], BF16)
                nc.sync.dma_start(out=s1h[:], in_=d_s1h[:, :].rearrange("(f p) n -> p f n", p=P))
                s1l = sw.tile([P, 16, 512], BF16)
                nc.sync.dma_start(out=s1l[:], in_=d_s1l[:, :].rearrange("(f p) n -> p f n", p=P))

                for c in range(NCHUNK):
                    xt_hi = sa.tile([P, 16, TC], BF16, tag="xt_hi")
                    nc.gpsimd.dma_gather(
                        out_ap=xt_hi[:], in_ap=d_xhi[:, :],
                        idxs_ap=gidx[:, c * (TC // 16):(c + 1) * (TC // 16)],
                        num_idxs=TC, num_idxs_reg=TC, elem_size=H, transpose=True)
                    xt_lo = sa.tile([P, 16, TC], BF16, tag="xt_lo")
                    nc.gpsimd.dma_gather(
                        out_ap=xt_lo[:], in_ap=d_xlo[:, :],
                        idxs_ap=gidx[:, c * (TC // 16):(c + 1) * (TC // 16)],
                        num_idxs=TC, num_idxs_reg=TC, elem_size=H, transpose=True)

                    # broadcast freq/imp rows across partitions via K=1 matmul
                    fps = sp1.tile([P, TC], F32, tag="fps")
                    nc.tensor.matmul(fps[:], onesf[:], freqr[:, c * TC:(c + 1) * TC],
                                     start=True, stop=True)
                    freqB = sa.tile([P, TC], F32, tag="freqB")
                    nc.vector.tensor_copy(freqB[:], fps[:])
                    ips = sp1.tile([P, TC], F32, tag="ips")
                    nc.tensor.matmul(ips[:], onesf[:], impr[:, c * TC:(c + 1) * TC],
                                     start=True, stop=True)
                    impB = sa.tile([P, TC], F32, tag="impB")
                    nc.vector.tensor_copy(impB[:], ips[:])

                    hid_hi = sa.tile([P, 4, TC], BF16, tag="hid_hi")
                    hid_lo = sa.tile([P, 4, TC], BF16, tag="hid_lo")
                    for j in range(4):
                        psA = sp.tile([P, TC], F32, tag="psA")
                        psB = sp.tile([P, TC], F32, tag="psB")
                        terms = [(s1h, xt_hi), (s1h, xt_lo), (s1l, xt_hi)]
                        n_mm = len(terms) * 8
                        i = 0
                        for (wsb, xsb) in terms:
                            for f in range(8):
                                fl = _mm_flags(i, n_mm)
                                nc.tensor.matmul(psA[:], wsb[:, f, j * P:(j + 1) * P],
                                                 xsb[:, f, :], **fl)
                                nc.tensor.matmul(psB[:], wsb[:, 8 + f, j * P:(j + 1) * P],
                                                 xsb[:, 8 + f, :], **fl)
                                i += 1
                        t0 = sa.tile([P, TC], F32, tag="t0")
                        nc.vector.tensor_tensor(t0[:], psA[:], freqB[:], op=mybir.AluOpType.mult)
                        t1 = sa.tile([P, TC], F32, tag="t1")
                        nc.vector.tensor_tensor(t1[:], psB[:], impB[:], op=mybir.AluOpType.mult)
                        t2 = sa.tile([P, TC], F32, tag="t2")
                        nc.vector.tensor_tensor(t2[:], t0[:], t1[:], op=mybir.AluOpType.add)
                        hidf = sa.tile([P, TC], F32, tag="hidf")
                        nc.scalar.activation(hidf[:], t2[:], mybir.ActivationFunctionType.Relu,
                                             bias=bs1[:, j:j + 1], scale=1.0)
                        nc.vector.tensor_copy(hid_hi[:, j, :], hidf[:])
                        nc.vector.tensor_tensor(hid_lo[:, j, :], hidf[:], hid_hi[:, j, :],
                                                op=mybir.AluOpType.subtract)

                    for m in range(TC // P):
                        psL = sp1.tile([P, 3], F32, tag="psL")
                        terms = [(hid_hi, s2h), (hid_lo, s2h), (hid_hi, s2l)]
                        i = 0
                        for (hsb, wsb) in terms:
                            for j in range(4):
                                nc.tensor.matmul(psL[:], hsb[:, j, m * P:(m + 1) * P],
                                                 wsb[:, j, :], **_mm_flags(i, 12))
                                i += 1
                        logits = sa.tile([P, 3], F32, tag="logits")
                        nc.vector.tensor_tensor(logits[:], psL[:], b2bc[:], op=mybir.AluOpType.add)
                        maxv = sa.tile([P, 1], F32, tag="maxv")
                        nc.vector.reduce_max(maxv[:], logits[:], axis=mybir.AxisListType.X)
                        mk = masks[:, c * (TC // P) + m, :]
                        e1 = sa.tile([P, 1], F32, tag="e1")
                        tt = sa.tile([P, 1], F32, tag="tt")
                        # m0 = (l0 == max)
                        nc.vector.tensor_scalar(mk[:, 0:1], logits[:, 0:1], maxv[:, 0:1], None,
                                                op0=mybir.AluOpType.is_equal)
                        # m1 = (l1 == max) * (1 - m0)
                        nc.vector.tensor_scalar(e1[:], logits[:, 1:2], maxv[:, 0:1], None,
                                                op0=mybir.AluOpType.is_equal)
                        nc.vector.tensor_tensor(tt[:], e1[:], mk[:, 0:1], op=mybir.AluOpType.mult)
                        nc.vector.tensor_tensor(mk[:, 1:2], e1[:], tt[:], op=mybir.AluOpType.subtract)
                        # m2 = 1 - m0 - m1
                        nc.vector.tensor_tensor(tt[:], mk[:, 0:1], mk[:, 1:2], op=mybir.AluOpType.add)
                        nc.vector.tensor_scalar(mk[:, 2:3], tt[:], -1.0, 1.0,
                                                op0=mybir.AluOpType.mult, op1=mybir.AluOpType.add)

            # ---------------- pass 2: branches + select ----------------
            with tc.tile_pool(name="brw", bufs=1) as bw, \
                 tc.tile_pool(name="bract", bufs=2) as ba, \
                 tc.tile_pool(name="brps", bufs=2, space="PSUM") as bp, \
                 tc.tile_pool(name="brps1", bufs=1, space="PSUM") as bp1:
                wc1 = bw.tile([P, 16, 1024], BF16)
                nc.sync.dma_start(out=wc1[:], in_=d_wc1[:, :].rearrange("(f p) n -> p f n", p=P))
                wa1 = bw.tile([P, 8, 1024], BF16)
                nc.sync.dma_start(out=wa1[:], in_=d_wa1[:, :].rearrange("(f p) n -> p f n", p=P))
                wd1 = bw.tile([P, 8, H], BF16)
                nc.sync.dma_start(out=wd1[:], in_=d_wd1[:, :].rearrange("(f p) n -> p f n", p=P))
                wc2 = bw.tile([P, 16, 512], BF16)
                nc.sync.dma_start(out=wc2[:], in_=d_wc2[:, :].rearrange("(f p) n -> p f n", p=P))
                wa2 = bw.tile([P, 4, 512], BF16)
                nc.sync.dma_start(out=wa2[:], in_=d_wa2[:, :].rearrange("(f p) n -> p f n", p=P))
                wd2 = bw.tile([P, 4, H], BF16)
                nc.sync.dma_start(out=wd2[:], in_=d_wd2[:, :].rearrange("(f p) n -> p f n", p=P))

                for c in range(NCHUNK):
                    xt = ba.tile([P, 16, TC], BF16, tag="xt")
                    nc.gpsimd.dma_gather(
                        out_ap=xt[:], in_ap=d_xhi[:, :],
                        idxs_ap=gidx[:, c * (TC // 16):(c + 1) * (TC // 16)],
                        num_idxs=TC, num_idxs_reg=TC, elem_size=H, transpose=True)

                    c1 = ba.tile([P, 8, TC], BF16, tag="c1")
                    for j in range(8):
                        ps = bp.tile([P, TC], F32, tag="psc")
                        for f in range(16):
                            nc.tensor.matmul(ps[:], wc1[:, f, j * P:(j + 1) * P],
                                             xt[:, f, :], **_mm_flags(f, 16))
                        nc.scalar.copy(c1[:, j, :], ps[:])
                    a1 = ba.tile([P, 8, TC], BF16, tag="a1")
                    for j in range(8):
                        ps = bp.tile([P, TC], F32, tag="psc")
                        for f in range(8):
                            nc.tensor.matmul(ps[:], wa1[:, f, j * P:(j + 1) * P],
                                             c1[:, f, :], **_mm_flags(f, 8))
                        nc.scalar.copy(a1[:, j, :], ps[:])
                    c2 = ba.tile([P, 4, TC], BF16, tag="c2")
                    for j in range(4):
                        ps = bp.tile([P, TC], F32, tag="psc")
                        for f in range(16):
                            nc.tensor.matmul(ps[:], wc2[:, f, j * P:(j + 1) * P],
                                             xt[:, f, :], **_mm_flags(f, 16))
                        nc.scalar.copy(c2[:, j, :], ps[:])
                    a2 = ba.tile([P, 4, TC], BF16, tag="a2")
                    for j in range(4):
                        ps = bp.tile([P, TC], F32, tag="psc")
                        for f in range(4):
                            nc.tensor.matmul(ps[:], wa2[:, f, j * P:(j + 1) * P],
                                             c2[:, f, :], **_mm_flags(f, 4))
                        nc.scalar.copy(a2[:, j, :], ps[:])

                    for m in range(TC // P):
                        mk = masks[:, c * (TC // P) + m, :]
                        for n in range(4):
                            ps1 = bp.tile([P, 512], F32, tag="ps1")
                            for f in range(8):
                                nc.tensor.matmul(ps1[:], a1[:, f, m * P:(m + 1) * P],
                                                 wd1[:, f, n * 512:(n + 1) * 512],
                                                 start=(f == 0), stop=False)
                            nc.tensor.matmul(ps1[:], onesb[:], be1[:, n * 512:(n + 1) * 512],
                                             start=False, stop=True)
                            ps2 = bp.tile([P, 512], F32, tag="ps2")
                            for f in range(4):
                                nc.tensor.matmul(ps2[:], a2[:, f, m * P:(m + 1) * P],
                                                 wd2[:, f, n * 512:(n + 1) * 512],
                                                 start=(f == 0), stop=False)
                            nc.tensor.matmul(ps2[:], onesb[:], be2[:, n * 512:(n + 1) * 512],
                                             start=False, stop=True)
                            # token-major x via PE transpose (x = xt.T @ I)
                            psx = bp1.tile([P, 512], F32, tag="psx")
                            for i in range(4):
                                nc.tensor.matmul(psx[:, i * P:(i + 1) * P],
                                                 xt[:, n * 4 + i, m * P:(m + 1) * P],
                                                 ident[:], start=True, stop=True)
                            selo = ba.tile([P, 512], F32, tag="selo")
                            nc.scalar.mul(selo[:], psx[:], mk[:, 0:1])
                            u1 = ba.tile([P, 512], F32, tag="u1")
                            nc.vector.tensor_scalar(u1[:], ps1[:], mk[:, 1:2], None,
                                                    op0=mybir.AluOpType.mult)
                            u2 = ba.tile([P, 512], F32, tag="u2")
                            nc.vector.tensor_scalar(u2[:], ps2[:], mk[:, 2:3], None,
                                                    op0=mybir.AluOpType.mult)
                            nc.vector.tensor_tensor(selo[:], selo[:], u1[:], op=mybir.AluOpType.add)
                            nc.vector.tensor_tensor(selo[:], selo[:], u2[:], op=mybir.AluOpType.add)
                            r0 = c * TC + m * P
                            nc.sync.dma_start(out=d_out[r0:r0 + P, n * 512:(n + 1) * 512],
                                              in_=selo[:])

    nc.finalize()
    return nc


_cached = {}


def _prep_shared(comp1_W, comp1_b, adapt1_W, adapt1_b, decomp1_W, decomp1_b,
                 comp2_W, comp2_b, adapt2_W, adapt2_b, decomp2_W, decomp2_b,
                 sel1_W, sel1_b, sel2_W, sel2_b):
    f32 = np.float32
    sel1_W = np.asarray(sel1_W, f32)
    sel2_W = np.asarray(sel2_W, f32)
    s1T = np.ascontiguousarray(sel1_W.T)           # [H, 512]
    s1h = _bf(s1T)
    s1l = _bf(s1T - s1h.astype(f32))
    s2T = np.ascontiguousarray(sel2_W.T)           # [512, 3]
    s2h = _bf(s2T)
    s2l = _bf(s2T - s2h.astype(f32))

    be1 = (np.asarray(decomp1_W, f32) @ (np.asarray(adapt1_W, f32) @ np.asarray(comp1_b, f32)
           + np.asarray(adapt1_b, f32)) + np.asarray(decomp1_b, f32))
    be2 = (np.asarray(decomp2_W, f32) @ (np.asarray(adapt2_W, f32) @ np.asarray(comp2_b, f32)
           + np.asarray(adapt2_b, f32)) + np.asarray(decomp2_b, f32))

    gidx = np.zeros((16, T // 16), np.int16)
    for i in range(T):
        gidx[i % 16, i // 16] = i
    gidx = np.tile(gidx, (8, 1))

    shared = {
        "ws1h": s1h, "ws1l": s1l,
        "ws2h": s2h, "ws2l": s2l,
        "wc1": _bf(np.asarray(comp1_W, f32).T),
        "wa1": _bf(np.asarray(adapt1_W, f32).T),
        "wd1": _bf(np.asarray(decomp1_W, f32).T),
        "wc2": _bf(np.asarray(comp2_W, f32).T),
        "wa2": _bf(np.asarray(adapt2_W, f32).T),
        "wd2": _bf(np.asarray(decomp2_W, f32).T),
        "bs1": np.asarray(sel1_b, f32).reshape(4, P).T.copy(),
        "b2bc": np.tile(np.asarray(sel2_b, f32).reshape(1, 3), (P, 1)),
        "be1": _bf(be1).reshape(1, H),
        "be2": _bf(be2).reshape(1, H),
        "onesb": np.ones((1, P), ml_dtypes.bfloat16),
        "onesf": np.ones((1, P), np.float32),
        "ident": np.eye(P, dtype=ml_dtypes.bfloat16),
        "gidx": gidx,
    }
    for k, v in shared.items():
        shared[k] = np.ascontiguousarray(v)
    return shared


def kernel(hidden_states, access_frequency, importance_score,
           comp1_W, comp1_b, adapt1_W, adapt1_b, decomp1_W, decomp1_b,
           comp2_W, comp2_b, adapt2_W, adapt2_b, decomp2_W, decomp2_b,
           sel1_W, sel1_b, sel2_W, sel2_b):
    global last_exec_time_ns
    f32 = np.float32
    hs = np.asarray(hidden_states, f32)
    B, S, _H = hs.shape
    x = hs.reshape(-1, _H)
    freq = np.asarray(access_frequency, f32).reshape(-1)
    imp = np.asarray(importance_score, f32).reshape(-1)

    shared = _prep_shared(comp1_W, comp1_b, adapt1_W, adapt1_b, decomp1_W, decomp1_b,
                          comp2_W, comp2_b, adapt2_W, adapt2_b, decomp2_W, decomp2_b,
                          sel1_W, sel1_b, sel2_W, sel2_b)

    xhi = x.astype(ml_dtypes.bfloat16)
    xlo = (x - xhi.astype(f32)).astype(ml_dtypes.bfloat16)

    in_maps = []
    for c in range(NCORES):
        sl = slice(c * T, (c + 1) * T)
        m = dict(shared)
        m["xhi"] = np.ascontiguousarray(xhi[sl])
        m["xlo"] = np.ascontiguousarray(xlo[sl])
        m["freqr"] = np.ascontiguousarray(freq[sl].reshape(1, T))
        m["impr"] = np.ascontiguousarray(imp[sl].reshape(1, T))
        in_maps.append(m)

    if "nc" not in _cached:
        _cached["nc"] = build_nc()
    nc = _cached["nc"]

    trace = os.environ.get("KERNEL_TRACE", "1") == "1"
    res = run_bass_kernel_spmd(nc, in_maps, core_ids=list(range(NCORES)), trace=trace)
    last_exec_time_ns = res.exec_time_ns
    if res.exec_time_ns is not None:
        print(f"HW exec time: {res.exec_time_ns} ns")

    out = np.concatenate([res.results[c]["out"] for c in range(NCORES)], axis=0)
    return out.reshape(B, S, _H).astype(np.float32)


# revision 5
# speedup vs baseline: 1.1831x; 1.1831x over previous
"""AdaptiveMemoryBank kernel for 8 TRN2 NeuronCores.

Data-parallel over tokens: B*S = 16384 tokens are split into 8 shards of
2048 rows; each core holds the full weight set and computes its shard.

Per-core pipeline (feature-major activation spine, bf16 matmuls):
  pass 1 (selector): XT = gather-transpose(x); hid = relu(scale-mixed
    split-bf16 matmul); logits = split-bf16 matmul -> argmax masks.
    split-bf16 (x_hi@w_hi + x_lo@w_hi + x_hi@w_lo) reproduces the fp32
    argmax exactly (0 flips on the graded input) at bf16 speed.
  pass 2 (branches): c = C@xT, a = A@cT (feature-major), out_k = aT.T@D
    (token-major) with all per-layer biases folded into one effective
    bias per branch (b_eff = D(A bc + ba) + bd) applied via a K=1 matmul;
    per-token select out = m0*x + m1*out1 + m2*out2 with masks as
    per-partition scalars.
"""

import sys, os, types, time

sys.path.insert(0, "/opt/trn_rl_repo")

# Provide the antenv.axon_hooks module the container's antenv stub lacks so
# run_bass_kernel_spmd(trace=True) can capture NTFF timing through axon.
if "antenv.axon_hooks" not in sys.modules:
    _hooks_mod = types.ModuleType("antenv.axon_hooks")
    _hooks_mod._hook = None

    def _set_hook(h):
        _hooks_mod._hook = h

    def _get_hook():
        return _hooks_mod._hook

    _hooks_mod.set_axon_ntff_profile_hook = _set_hook
    _hooks_mod.get_axon_ntff_profile_hook = _get_hook
    sys.modules["antenv.axon_hooks"] = _hooks_mod
    try:
        from trn_agent_boot.trn_boot import _ntff_profile_via_ctypes

        _set_hook(_ntff_profile_via_ctypes("/opt/axon/libaxon_pjrt.so"))
    except Exception:
        pass

import numpy as np
import ml_dtypes

import concourse.bass as bass
import concourse.bacc as bacc
import concourse.tile as tile
import concourse.mybir as mybir
from concourse.bass_utils import run_bass_kernel_spmd

BF16 = mybir.dt.bfloat16
F32 = mybir.dt.float32
I16 = mybir.dt.int16

NCORES = 8
H = 2048
T = 2048          # tokens per core
TC = 256          # tokens per chunk
NCHUNK = T // TC
P = 128

last_exec_time_ns = None
last_results = None


def _bf(x):
    return np.asarray(x, np.float32).astype(ml_dtypes.bfloat16)


def _mm_flags(i, n):
    return dict(start=(i == 0), stop=(i == n - 1))


def build_nc():
    nc = bacc.Bacc(None, target_bir_lowering=False, debug=False)

    d_xhi = nc.dram_tensor("xhi", [T, H], BF16, kind="ExternalInput")
    d_xlo = nc.dram_tensor("xlo", [T, H], BF16, kind="ExternalInput")
    d_freq = nc.dram_tensor("freqr", [1, T], F32, kind="ExternalInput")
    d_imp = nc.dram_tensor("impr", [1, T], F32, kind="ExternalInput")
    d_s1h = nc.dram_tensor("ws1h", [H, 512], BF16, kind="ExternalInput")
    d_s1l = nc.dram_tensor("ws1l", [H, 512], BF16, kind="ExternalInput")
    d_s2h = nc.dram_tensor("ws2h", [512, 3], BF16, kind="ExternalInput")
    d_s2l = nc.dram_tensor("ws2l", [512, 3], BF16, kind="ExternalInput")
    d_wc1 = nc.dram_tensor("wc1", [H, 1024], BF16, kind="ExternalInput")
    d_wa1 = nc.dram_tensor("wa1", [1024, 1024], BF16, kind="ExternalInput")
    d_wd1 = nc.dram_tensor("wd1", [1024, H], BF16, kind="ExternalInput")
    d_wc2 = nc.dram_tensor("wc2", [H, 512], BF16, kind="ExternalInput")
    d_wa2 = nc.dram_tensor("wa2", [512, 512], BF16, kind="ExternalInput")
    d_wd2 = nc.dram_tensor("wd2", [512, H], BF16, kind="ExternalInput")
    d_bs1 = nc.dram_tensor("bs1", [P, 4], F32, kind="ExternalInput")
    d_b2bc = nc.dram_tensor("b2bc", [P, 3], F32, kind="ExternalInput")
    d_be1 = nc.dram_tensor("be1", [1, H], BF16, kind="ExternalInput")
    d_be2 = nc.dram_tensor("be2", [1, H], BF16, kind="ExternalInput")
    d_ones = nc.dram_tensor("onesb", [1, P], BF16, kind="ExternalInput")
    d_onesf = nc.dram_tensor("onesf", [1, P], F32, kind="ExternalInput")
    d_ident = nc.dram_tensor("ident", [P, P], BF16, kind="ExternalInput")
    d_gidx = nc.dram_tensor("gidx", [P, T // 16], I16, kind="ExternalInput")
    d_out = nc.dram_tensor("out", [T, H], F32, kind="ExternalOutput")

    with tile.TileContext(nc) as tc:
        with tc.tile_pool(name="persist", bufs=1) as pp:
            gidx = pp.tile([P, T // 16], I16)
            nc.sync.dma_start(out=gidx[:], in_=d_gidx[:, :])
            onesb = pp.tile([1, P], BF16)
            nc.sync.dma_start(out=onesb[:], in_=d_ones[:, :])
            onesf = pp.tile([1, P], F32)
            nc.sync.dma_start(out=onesf[:], in_=d_onesf[:, :])
            ident = pp.tile([P, P], BF16)
            nc.sync.dma_start(out=ident[:], in_=d_ident[:, :])
            bs1 = pp.tile([P, 4], F32)
            nc.sync.dma_start(out=bs1[:], in_=d_bs1[:, :])
            b2bc = pp.tile([P, 3], F32)
            nc.sync.dma_start(out=b2bc[:], in_=d_b2bc[:, :])
            be1 = pp.tile([1, H], BF16)
            nc.sync.dma_start(out=be1[:], in_=d_be1[:, :])
            be2 = pp.tile([1, H], BF16)
            nc.sync.dma_start(out=be2[:], in_=d_be2[:, :])
            freqr = pp.tile([1, T], F32)
            nc.sync.dma_start(out=freqr[:], in_=d_freq[:, :])
            impr = pp.tile([1, T], F32)
            nc.sync.dma_start(out=impr[:], in_=d_imp[:, :])
            s2h = pp.tile([P, 4, 3], BF16)
            nc.sync.dma_start(out=s2h[:], in_=d_s2h[:, :].rearrange("(f p) n -> p f n", p=P))
            s2l = pp.tile([P, 4, 3], BF16)
            nc.sync.dma_start(out=s2l[:], in_=d_s2l[:, :].rearrange("(f p) n -> p f n", p=P))
            # per-token one-hot expert masks, [128, tok_tile, 3] f32
            masks = pp.tile([P, T // P, 3], F32)

            # ---------------- pass 1: selector ----------------
            with tc.tile_pool(name="selw", bufs=1) as sw, \
                 tc.tile_pool(name="selact", bufs=2) as sa, \
                 tc.tile_pool(name="selps", bufs=2, space="PSUM") as sp, \
                 tc.tile_pool(name="selps1", bufs=1, space="PSUM") as sp1:
                s1h = sw.tile([P, 16, 512], BF16)
                nc.sync.dma_start(out=s1h[:], in_=d_s1h[:, :].rearrange("(f p) n -> p f n", p=P))
                s1l = sw.tile([P, 16, 512], BF16)
                nc.sync.dma_start(out=s1l[:], in_=d_s1l[:, :].rearrange("(f p) n -> p f n", p=P))

                for c in range(NCHUNK):
                    xt_hi = sa.tile([P, 16, TC], BF16, tag="xt_hi")
                    nc.gpsimd.dma_gather(
                        out_ap=xt_hi[:], in_ap=d_xhi[:, :],
                        idxs_ap=gidx[:, c * (TC // 16):(c + 1) * (TC // 16)],
                        num_idxs=TC, num_idxs_reg=TC, elem_size=H, transpose=True)
                    xt_lo = sa.tile([P, 16, TC], BF16, tag="xt_lo")
                    nc.gpsimd.dma_gather(
                        out_ap=xt_lo[:], in_ap=d_xlo[:, :],
                        idxs_ap=gidx[:, c * (TC // 16):(c + 1) * (TC // 16)],
                        num_idxs=TC, num_idxs_reg=TC, elem_size=H, transpose=True)

                    # broadcast freq/imp rows across partitions via K=1 matmul
                    fps = sp1.tile([P, TC], F32, tag="fps")
                    nc.tensor.matmul(fps[:], onesf[:], freqr[:, c * TC:(c + 1) * TC],
                                     start=True, stop=True)
                    freqB = sa.tile([P, TC], F32, tag="freqB")
                    nc.vector.tensor_copy(freqB[:], fps[:])
                    ips = sp1.tile([P, TC], F32, tag="ips")
                    nc.tensor.matmul(ips[:], onesf[:], impr[:, c * TC:(c + 1) * TC],
                                     start=True, stop=True)
                    impB = sa.tile([P, TC], F32, tag="impB")
                    nc.vector.tensor_copy(impB[:], ips[:])

                    hid_hi = sa.tile([P, 4, TC], BF16, tag="hid_hi")
                    hid_lo = sa.tile([P, 4, TC], BF16, tag="hid_lo")
                    for j in range(4):
                        psA = sp.tile([P, TC], F32, tag="psA")
                        psB = sp.tile([P, TC], F32, tag="psB")
                        terms = [(s1h, xt_hi), (s1h, xt_lo), (s1l, xt_hi)]
                        n_mm = len(terms) * 8
                        i = 0
                        for (wsb, xsb) in terms:
                            for f in range(8):
                                fl = _mm_flags(i, n_mm)
                                nc.tensor.matmul(psA[:], wsb[:, f, j * P:(j + 1) * P],
                                                 xsb[:, f, :], **fl)
                                nc.tensor.matmul(psB[:], wsb[:, 8 + f, j * P:(j + 1) * P],
                                                 xsb[:, 8 + f, :], **fl)
                                i += 1
                        t0 = sa.tile([P, TC], F32, tag="t0")
                        nc.vector.tensor_tensor(t0[:], psA[:], freqB[:], op=mybir.AluOpType.mult)
                        t1 = sa.tile([P, TC], F32, tag="t1")
                        nc.vector.tensor_tensor(t1[:], psB[:], impB[:], op=mybir.AluOpType.mult)
                        t2 = sa.tile([P, TC], F32, tag="t2")
                        nc.vector.tensor_tensor(t2[:], t0[:], t1[:], op=mybir.AluOpType.add)
                        hidf = sa.tile([P, TC], F32, tag="hidf")
                        nc.scalar.activation(hidf[:], t2[:], mybir.ActivationFunctionType.Relu,
                                             bias=bs1[:, j:j + 1], scale=1.0)
                        nc.vector.tensor_copy(hid_hi[:, j, :], hidf[:])
                        nc.vector.tensor_tensor(hid_lo[:, j, :], hidf[:], hid_hi[:, j, :],
                                                op=mybir.AluOpType.subtract)

                    for m in range(TC // P):
                        psL = sp1.tile([P, 3], F32, tag="psL")
                        terms = [(hid_hi, s2h), (hid_lo, s2h), (hid_hi, s2l)]
                        i = 0
                        for (hsb, wsb) in terms:
                            for j in range(4):
                                nc.tensor.matmul(psL[:], hsb[:, j, m * P:(m + 1) * P],
                                                 wsb[:, j, :], **_mm_flags(i, 12))
                                i += 1
                        logits = sa.tile([P, 3], F32, tag="logits")
                        nc.vector.tensor_tensor(logits[:], psL[:], b2bc[:], op=mybir.AluOpType.add)
                        maxv = sa.tile([P, 1], F32, tag="maxv")
                        nc.vector.reduce_max(maxv[:], logits[:], axis=mybir.AxisListType.X)
                        mk = masks[:, c * (TC // P) + m, :]
                        e1 = sa.tile([P, 1], F32, tag="e1")
                        tt = sa.tile([P, 1], F32, tag="tt")
                        # m0 = (l0 == max)
                        nc.vector.tensor_scalar(mk[:, 0:1], logits[:, 0:1], maxv[:, 0:1], None,
                                                op0=mybir.AluOpType.is_equal)
                        # m1 = (l1 == max) * (1 - m0)
                        nc.vector.tensor_scalar(e1[:], logits[:, 1:2], maxv[:, 0:1], None,
                                                op0=mybir.AluOpType.is_equal)
                        nc.vector.tensor_tensor(tt[:], e1[:], mk[:, 0:1], op=mybir.AluOpType.mult)
                        nc.vector.tensor_tensor(mk[:, 1:2], e1[:], tt[:], op=mybir.AluOpType.subtract)
                        # m2 = 1 - m0 - m1
                        nc.vector.tensor_tensor(tt[:], mk[:, 0:1], mk[:, 1:2], op=mybir.AluOpType.add)
                        nc.vector.tensor_scalar(mk[:, 2:3], tt[:], -1.0, 1.0,
                                                op0=mybir.AluOpType.mult, op1=mybir.AluOpType.add)

            # ---------------- pass 2: branches + select ----------------
            with tc.tile_pool(name="brw", bufs=1) as bw, \
                 tc.tile_pool(name="bract", bufs=2) as ba, \
                 tc.tile_pool(name="brps", bufs=2, space="PSUM") as bp, \
                 tc.tile_pool(name="brps1", bufs=1, space="PSUM") as bp1:
                wc1 = bw.tile([P, 16, 1024], BF16)
                nc.sync.dma_start(out=wc1[:], in_=d_wc1[:, :].rearrange("(f p) n -> p f n", p=P))
                wa1 = bw.tile([P, 8, 1024], BF16)
                nc.sync.dma_start(out=wa1[:], in_=d_wa1[:, :].rearrange("(f p) n -> p f n", p=P))
                wd1 = bw.tile([P, 8, H], BF16)
                nc.sync.dma_start(out=wd1[:], in_=d_wd1[:, :].rearrange("(f p) n -> p f n", p=P))
                wc2 = bw.tile([P, 16, 512], BF16)
                nc.sync.dma_start(out=wc2[:], in_=d_wc2[:, :].rearrange("(f p) n -> p f n", p=P))
                wa2 = bw.tile([P, 4, 512], BF16)
                nc.sync.dma_start(out=wa2[:], in_=d_wa2[:, :].rearrange("(f p) n -> p f n", p=P))
                wd2 = bw.tile([P, 4, H], BF16)
                nc.sync.dma_start(out=wd2[:], in_=d_wd2[:, :].rearrange("(f p) n -> p f n", p=P))

                for c in range(NCHUNK):
                    xt = ba.tile([P, 16, TC], BF16, tag="xt")
                    nc.gpsimd.dma_gather(
                        out_ap=xt[:], in_ap=d_xhi[:, :],
                        idxs_ap=gidx[:, c * (TC // 16):(c + 1) * (TC // 16)],
                        num_idxs=TC, num_idxs_reg=TC, elem_size=H, transpose=True)

                    c1 = ba.tile([P, 8, TC], BF16, tag="c1")
                    for j in range(8):
                        ps = bp.tile([P, TC], F32, tag="psc")
                        for f in range(16):
                            nc.tensor.matmul(ps[:], wc1[:, f, j * P:(j + 1) * P],
                                             xt[:, f, :], **_mm_flags(f, 16))
                        nc.scalar.copy(c1[:, j, :], ps[:])
                    a1 = ba.tile([P, 8, TC], BF16, tag="a1")
                    for j in range(8):
                        ps = bp.tile([P, TC], F32, tag="psc")
                        for f in range(8):
                            nc.tensor.matmul(ps[:], wa1[:, f, j * P:(j + 1) * P],
                                             c1[:, f, :], **_mm_flags(f, 8))
                        nc.scalar.copy(a1[:, j, :], ps[:])
                    c2 = ba.tile([P, 4, TC], BF16, tag="c2")
                    for j in range(4):
                        ps = bp.tile([P, TC], F32, tag="psc")
                        for f in range(16):
                            nc.tensor.matmul(ps[:], wc2[:, f, j * P:(j + 1) * P],
                                             xt[:, f, :], **_mm_flags(f, 16))
                        nc.scalar.copy(c2[:, j, :], ps[:])
                    a2 = ba.tile([P, 4, TC], BF16, tag="a2")
                    for j in range(4):
                        ps = bp.tile([P, TC], F32, tag="psc")
                        for f in range(4):
                            nc.tensor.matmul(ps[:], wa2[:, f, j * P:(j + 1) * P],
                                             c2[:, f, :], **_mm_flags(f, 4))
                        nc.scalar.copy(a2[:, j, :], ps[:])

                    for m in range(TC // P):
                        mk = masks[:, c * (TC // P) + m, :]
                        for n in range(4):
                            ps1 = bp.tile([P, 512], F32, tag="ps1")
                            for f in range(8):
                                nc.tensor.matmul(ps1[:], a1[:, f, m * P:(m + 1) * P],
                                                 wd1[:, f, n * 512:(n + 1) * 512],
                                                 start=(f == 0), stop=False)
                            nc.tensor.matmul(ps1[:], onesb[:], be1[:, n * 512:(n + 1) * 512],
                                             start=False, stop=True)
                            ps2 = bp.tile([P, 512], F32, tag="ps2")
                            for f in range(4):
                                nc.tensor.matmul(ps2[:], a2[:, f, m * P:(m + 1) * P],
                                                 wd2[:, f, n * 512:(n + 1) * 512],
                                                 start=(f == 0), stop=False)
                            nc.tensor.matmul(ps2[:], onesb[:], be2[:, n * 512:(n + 1) * 512],
                                             start=False, stop=True)
                            # token-major x via PE transpose (x = xt.T @ I)
                            psx = bp1.tile([P, 512], F32, tag="psx")
                            for i in range(4):
                                nc.tensor.matmul(psx[:, i * P:(i + 1) * P],
                                                 xt[:, n * 4 + i, m * P:(m + 1) * P],
                                                 ident[:], start=True, stop=True)
                            selo = ba.tile([P, 512], F32, tag="selo")
                            nc.scalar.mul(selo[:], psx[:], mk[:, 0:1])
                            u1 = ba.tile([P, 512], F32, tag="u1")
                            nc.vector.tensor_scalar(u1[:], ps1[:], mk[:, 1:2], None,
                                                    op0=mybir.AluOpType.mult)
                            u2 = ba.tile([P, 512], F32, tag="u2")
                            nc.vector.tensor_scalar(u2[:], ps2[:], mk[:, 2:3], None,
                                                    op0=mybir.AluOpType.mult)
                            nc.vector.tensor_tensor(selo[:], selo[:], u1[:], op=mybir.AluOpType.add)
                            nc.vector.tensor_tensor(selo[:], selo[:], u2[:], op=mybir.AluOpType.add)
                            r0 = c * TC + m * P
                            nc.sync.dma_start(out=d_out[r0:r0 + P, n * 512:(n + 1) * 512],
                                              in_=selo[:])

    nc.finalize()
    return nc


_cached = {}


def _prep_shared(comp1_W, comp1_b, adapt1_W, adapt1_b, decomp1_W, decomp1_b,
                 comp2_W, comp2_b, adapt2_W, adapt2_b, decomp2_W, decomp2_b,
                 sel1_W, sel1_b, sel2_W, sel2_b):
    f32 = np.float32
    sel1_W = np.asarray(sel1_W, f32)
    sel2_W = np.asarray(sel2_W, f32)
    s1T = np.ascontiguousarray(sel1_W.T)           # [H, 512]
    s1h = _bf(s1T)
    s1l = _bf(s1T - s1h.astype(f32))
    s2T = np.ascontiguousarray(sel2_W.T)           # [512, 3]
    s2h = _bf(s2T)
    s2l = _bf(s2T - s2h.astype(f32))

    be1 = (np.asarray(decomp1_W, f32) @ (np.asarray(adapt1_W, f32) @ np.asarray(comp1_b, f32)
           + np.asarray(adapt1_b, f32)) + np.asarray(decomp1_b, f32))
    be2 = (np.asarray(decomp2_W, f32) @ (np.asarray(adapt2_W, f32) @ np.asarray(comp2_b, f32)
           + np.asarray(adapt2_b, f32)) + np.asarray(decomp2_b, f32))

    gidx = np.zeros((16, T // 16), np.int16)
    for i in range(T):
        gidx[i % 16, i // 16] = i
    gidx = np.tile(gidx, (8, 1))

    shared = {
        "ws1h": s1h, "ws1l": s1l,
        "ws2h": s2h, "ws2l": s2l,
        "wc1": _bf(np.asarray(comp1_W, f32).T),
        "wa1": _bf(np.asarray(adapt1_W, f32).T),
        "wd1": _bf(np.asarray(decomp1_W, f32).T),
        "wc2": _bf(np.asarray(comp2_W, f32).T),
        "wa2": _bf(np.asarray(adapt2_W, f32).T),
        "wd2": _bf(np.asarray(decomp2_W, f32).T),
        "bs1": np.asarray(sel1_b, f32).reshape(4, P).T.copy(),
        "b2bc": np.tile(np.asarray(sel2_b, f32).reshape(1, 3), (P, 1)),
        "be1": _bf(be1).reshape(1, H),
        "be2": _bf(be2).reshape(1, H),
        "onesb": np.ones((1, P), ml_dtypes.bfloat16),
        "onesf": np.ones((1, P), np.float32),
        "ident": np.eye(P, dtype=ml_dtypes.bfloat16),
        "gidx": gidx,
    }
    for k, v in shared.items():
        shared[k] = np.ascontiguousarray(v)
    return shared


def kernel(hidden_states, access_frequency, importance_score,
           comp1_W, comp1_b, adapt1_W, adapt1_b, decomp1_W, decomp1_b,
           comp2_W, comp2_b, adapt2_W, adapt2_b, decomp2_W, decomp2_b,
           sel1_W, sel1_b, sel2_W, sel2_b):
    global last_exec_time_ns, last_results
    f32 = np.float32
    hs = np.asarray(hidden_states, f32)
    B, S, _H = hs.shape
    x = hs.reshape(-1, _H)
    freq = np.asarray(access_frequency, f32).reshape(-1)
    imp = np.asarray(importance_score, f32).reshape(-1)

    shared = _prep_shared(comp1_W, comp1_b, adapt1_W, adapt1_b, decomp1_W, decomp1_b,
                          comp2_W, comp2_b, adapt2_W, adapt2_b, decomp2_W, decomp2_b,
                          sel1_W, sel1_b, sel2_W, sel2_b)

    xhi = x.astype(ml_dtypes.bfloat16)
    xlo = (x - xhi.astype(f32)).astype(ml_dtypes.bfloat16)

    in_maps = []
    for c in range(NCORES):
        sl = slice(c * T, (c + 1) * T)
        m = dict(shared)
        m["xhi"] = np.ascontiguousarray(xhi[sl])
        m["xlo"] = np.ascontiguousarray(xlo[sl])
        m["freqr"] = np.ascontiguousarray(freq[sl].reshape(1, T))
        m["impr"] = np.ascontiguousarray(imp[sl].reshape(1, T))
        in_maps.append(m)

    if "nc" not in _cached:
        _cached["nc"] = build_nc()
    nc = _cached["nc"]

    trace = os.environ.get("KERNEL_TRACE", "1") == "1"
    res = run_bass_kernel_spmd(nc, in_maps, core_ids=list(range(NCORES)), trace=trace)
    last_results = res
    last_exec_time_ns = res.exec_time_ns
    if res.exec_time_ns is not None:
        print(f"HW exec time: {res.exec_time_ns} ns")

    out = np.concatenate([res.results[c]["out"] for c in range(NCORES)], axis=0)
    return out.reshape(B, S, _H).astype(np.float32)
